# revision 2
# baseline (speedup 1.0000x reference)
"""Single-head causal attention (B=8, S=2048, E=2048, D=128) on 8 trn2 cores.

Sharding: data-parallel over batch — one batch element per NeuronCore.

Per-core dataflow (all matmuls bf16 with f32 PSUM accumulation):
  - host supplies xT (E,S) = x[b].T and WT (E,D) = W.T so the contraction
    dim E lands on SBUF partitions
  - projections produce qT/kT/vT in [D, S] layout (bias fused into the
    ScalarE PSUM->SBUF evacuation)
  - vT is re-transposed on the PE into natural [S, D] blocks, augmented
    with a ones column (col 128): the AV matmul then yields the softmax
    denominator for free as output column 128
  - scoresT[k, q] per k-block j: single matmul (K=D=128), exact causal
    trim of the q range; diagonal 128-block masked by adding -1e30;
    ScalarE computes exp(scale*s) straight out of PSUM into bf16 probsT
  - AV per q-block i accumulates probsT_j.T @ v_aug_j over j<=i in PSUM;
    DVE takes 1/denominator, ScalarE applies it during the final
    evacuation, DMA stores the natural-layout [128, 128] result
"""

import math

import numpy as np

B = 8
S = 2048
E = 2048
D = 128
P = 128
NE = E // P  # 16 contraction chunks
NS = S // P  # 16 sequence blocks
ST = 512  # s-tile width for projections / score chunks
NST = S // ST  # 4
VW = D + 1  # v block width incl. ones column
SCALE = 1.0 / math.sqrt(S)
NEG = -1.0e30

_PROGRAM = None


def build_program():
    global _PROGRAM
    if _PROGRAM is not None:
        return _PROGRAM

    import concourse.bacc as bacc
    import concourse.mybir as mybir
    import concourse.tile as tile
    from concourse.masks import make_identity

    f32 = mybir.dt.float32
    bf16 = mybir.dt.bfloat16

    nc = bacc.Bacc("TRN2", target_bir_lowering=False, debug=False)

    xT_d = nc.dram_tensor("xT", [E, S], f32, kind="ExternalInput")
    w_d = {
        "q": nc.dram_tensor("wqT", [E, D], f32, kind="ExternalInput"),
        "k": nc.dram_tensor("wkT", [E, D], f32, kind="ExternalInput"),
        "v": nc.dram_tensor("wvT", [E, D], f32, kind="ExternalInput"),
    }
    b_d = {
        "q": nc.dram_tensor("bq", [D, 1], f32, kind="ExternalInput"),
        "k": nc.dram_tensor("bk", [D, 1], f32, kind="ExternalInput"),
        "v": nc.dram_tensor("bv", [D, 1], f32, kind="ExternalInput"),
    }
    out_d = nc.dram_tensor("out", [S, D], f32, kind="ExternalOutput")

    with tile.TileContext(nc) as tc:
        with (
            tc.tile_pool(name="const", bufs=1) as cpool,
            tc.tile_pool(name="xt", bufs=1) as xpool,
            tc.tile_pool(name="qkv", bufs=1) as qkvpool,
            tc.tile_pool(name="probs", bufs=20) as ppool,
            tc.tile_pool(name="osb", bufs=2) as opool,
            tc.tile_pool(name="misc", bufs=2) as mpool,
            tc.tile_pool(name="proj_ps", bufs=3, space="PSUM") as proj_ps,
            tc.tile_pool(name="sc_ps", bufs=2, space="PSUM") as sc_ps,
            tc.tile_pool(name="vt_ps", bufs=1, space="PSUM") as vt_ps,
            tc.tile_pool(name="out_ps", bufs=2, space="PSUM") as out_ps,
        ):
            # ---- constants ----
            ident = cpool.tile([P, P], bf16, tag="ident")
            make_identity(nc, ident[:])
            # cmaskT[k_local, q_local]: 0 where q >= k (valid), -1e30 where q < k
            cmaskT = cpool.tile([P, P], f32, tag="cmaskT")
            nc.gpsimd.memset(cmaskT[:], 0.0)
            nc.gpsimd.affine_select(
                out=cmaskT[:],
                in_=cmaskT[:],
                compare_op=mybir.AluOpType.is_ge,
                fill=NEG,
                base=0,
                # iota[r, c] = c - r ; keep (0.0) where c - r >= 0
                pattern=[[1, P]],
                channel_multiplier=-1,
            )

            w_sb = {}
            b_sb = {}
            for pj in ("q", "k", "v"):
                w_sb[pj] = cpool.tile([P, NE * D], bf16, name=f"w{pj}", tag=f"w{pj}")
                nc.gpsimd.dma_start(
                    w_sb[pj][:].rearrange("p (ec d) -> p ec d", ec=NE),
                    w_d[pj].rearrange("(ec p) d -> p ec d", p=P),
                )
                b_sb[pj] = cpool.tile([P, 1], f32, name=f"b{pj}", tag=f"b{pj}")
                nc.sync.dma_start(b_sb[pj][:], b_d[pj][:, :])

            # ---- xT load (f32 -> bf16 cast on SWDGE), s-tile-major ----
            xt_sb = xpool.tile([P, NE * S], bf16, tag="xt")
            for st in range(NST):
                nc.gpsimd.dma_start(
                    xt_sb[:].rearrange("p (ec s) -> p ec s", ec=NE)[
                        :, :, st * ST : (st + 1) * ST
                    ],
                    xT_d.rearrange("(ec p) s -> p ec s", p=P)[
                        :, :, st * ST : (st + 1) * ST
                    ],
                )

            qT_sb = qkvpool.tile([P, S], bf16, tag="qT")
            kT_sb = qkvpool.tile([P, S], bf16, tag="kT")
            vT_sb = qkvpool.tile([P, S], bf16, tag="vT")
            v_sb = qkvpool.tile([P, NS * VW], bf16, tag="v")
            dest = {"q": qT_sb, "k": kT_sb, "v": vT_sb}

            # ones column of v_aug
            for sb in range(NS):
                nc.vector.memset(v_sb[:, sb * VW + D : (sb + 1) * VW], 1.0)

            probs_pieces = {}

            for st in range(NST):
                # ---- projections for this s-tile ----
                for pj in ("q", "k", "v"):
                    ps = proj_ps.tile([P, ST], f32, tag="proj")
                    for ec in range(NE):
                        nc.tensor.matmul(
                            ps[:],
                            lhsT=w_sb[pj][:, ec * D : (ec + 1) * D],
                            rhs=xt_sb[:, ec * S + st * ST : ec * S + (st + 1) * ST],
                            start=(ec == 0),
                            stop=(ec == NE - 1),
                        )
                    nc.scalar.activation(
                        dest[pj][:, st * ST : (st + 1) * ST],
                        ps[:],
                        func=mybir.ActivationFunctionType.Identity,
                        bias=b_sb[pj][:, 0:1],
                        scale=1.0,
                    )

                # ---- v natural blocks (PE transpose of vT) ----
                for sb in range(st * (ST // P), (st + 1) * (ST // P)):
                    tp = vt_ps.tile([P, P], bf16, tag="vt")
                    nc.tensor.transpose(
                        tp[:], vT_sb[:, sb * P : (sb + 1) * P], ident[:]
                    )
                    nc.vector.tensor_copy(v_sb[:, sb * VW : sb * VW + D], tp[:])

                # ---- scoresT + exp for q-chunk c = st ----
                c = st
                for j in range(4 * c + 4):
                    qs = max(c * ST, j * P)
                    w = (c + 1) * ST - qs
                    sps = sc_ps.tile([P, ST], f32, tag="sc")
                    nc.tensor.matmul(
                        sps[:, :w],
                        lhsT=kT_sb[:, j * P : (j + 1) * P],
                        rhs=qT_sb[:, qs : qs + w],
                        start=True,
                        stop=True,
                    )
                    if j * P >= c * ST:
                        # diagonal block occupies the first 128 columns
                        nc.vector.tensor_add(sps[:, 0:P], sps[:, 0:P], cmaskT[:])
                    prb = ppool.tile([P, ST], bf16, tag="probs")
                    nc.scalar.activation(
                        prb[:, :w],
                        sps[:, :w],
                        func=mybir.ActivationFunctionType.Exp,
                        bias=0.0,
                        scale=SCALE,
                    )
                    probs_pieces[(j, c)] = (prb, qs)

                # ---- AV + normalize + store for the 4 q-blocks of chunk c ----
                for i in range(4 * c, 4 * c + 4):
                    ops = out_ps.tile([P, VW], f32, tag="out")
                    for j in range(i + 1):
                        prb, qs = probs_pieces[(j, c)]
                        off = i * P - qs
                        nc.tensor.matmul(
                            ops[:],
                            lhsT=prb[:, off : off + P],
                            rhs=v_sb[:, j * VW : (j + 1) * VW],
                            start=(j == 0),
                            stop=(j == i),
                        )
                    recip = mpool.tile([P, 1], f32, tag="recip")
                    nc.vector.reciprocal(recip[:], ops[:, D : D + 1])
                    osb = opool.tile([P, D], f32, tag="osb")
                    nc.scalar.activation(
                        osb[:],
                        ops[:, 0:D],
                        func=mybir.ActivationFunctionType.Copy,
                        bias=0.0,
                        scale=recip[:, 0:1],
                    )
                    nc.sync.dma_start(out_d[i * P : (i + 1) * P, :], osb[:])

    nc.compile()
    _PROGRAM = nc
    return nc


def make_in_maps(x, Wq, bq, Wk, bk, Wv, bv):
    x = np.asarray(x, dtype=np.float32)
    shared = {
        "wqT": np.ascontiguousarray(np.asarray(Wq, dtype=np.float32).T),
        "wkT": np.ascontiguousarray(np.asarray(Wk, dtype=np.float32).T),
        "wvT": np.ascontiguousarray(np.asarray(Wv, dtype=np.float32).T),
        "bq": np.asarray(bq, dtype=np.float32).reshape(D, 1).copy(),
        "bk": np.asarray(bk, dtype=np.float32).reshape(D, 1).copy(),
        "bv": np.asarray(bv, dtype=np.float32).reshape(D, 1).copy(),
    }
    return [
        {"xT": np.ascontiguousarray(x[b].T), **shared} for b in range(B)
    ]


def kernel(x, Wq, bq, Wk, bk, Wv, bv):
    from concourse.bass_utils import run_bass_kernel_spmd

    nc = build_program()
    in_maps = make_in_maps(x, Wq, bq, Wk, bk, Wv, bv)
    res = run_bass_kernel_spmd(nc, in_maps, list(range(B)))
    return np.stack([res.results[i]["out"] for i in range(B)], axis=0)


# revision 5
# speedup vs baseline: 75.9380x; 75.9380x over previous
"""Single-head causal attention (B=8, S=2048, E=2048, D=128) on 8 trn2 cores.

Sharding: data-parallel over batch — one batch element per NeuronCore.

Per-core dataflow (all matmuls bf16 with f32 PSUM accumulation):
  - host supplies xT (E,S) = x[b].T and WT (E,D) = W.T so the contraction
    dim E lands on SBUF partitions
  - projections produce qT/kT/vT in [D, S] layout (bias fused into the
    ScalarE PSUM->SBUF evacuation)
  - vT is re-transposed on the PE into natural [S, D] blocks, augmented
    with a ones column (col 128): the AV matmul then yields the softmax
    denominator for free as output column 128
  - scoresT[k, q] per k-block j: single matmul (K=D=128), exact causal
    trim of the q range; diagonal 128-block masked by adding -1e30;
    ScalarE computes exp(scale*s) straight out of PSUM into bf16 probsT
  - AV per q-block i accumulates probsT_j.T @ v_aug_j over j<=i in PSUM;
    DVE takes 1/denominator, ScalarE applies it during the final
    evacuation, DMA stores the natural-layout [128, 128] result
"""

import math

import numpy as np

B = 8
S = 2048
E = 2048
D = 128
P = 128
NE = E // P  # 16 contraction chunks
NS = S // P  # 16 sequence blocks
ST = 512  # s-tile width for projections / score chunks
NST = S // ST  # 4
VW = D + 1  # v block width incl. ones column
SCALE = 1.0 / math.sqrt(S)
NEG = -1.0e30

_PROGRAMS = {}


def build_program(iters=1):
    global _PROGRAMS
    if iters in _PROGRAMS:
        return _PROGRAMS[iters]

    import contextlib

    import concourse.bacc as bacc
    import concourse.mybir as mybir
    import concourse.tile as tile
    from concourse.masks import make_identity

    f32 = mybir.dt.float32
    bf16 = mybir.dt.bfloat16

    nc = bacc.Bacc("TRN2", target_bir_lowering=False, debug=False)

    xT_d = nc.dram_tensor("xT", [E, S], f32, kind="ExternalInput")
    w_d = {
        "q": nc.dram_tensor("wqT", [E, D], f32, kind="ExternalInput"),
        "k": nc.dram_tensor("wkT", [E, D], f32, kind="ExternalInput"),
        "v": nc.dram_tensor("wvT", [E, D], f32, kind="ExternalInput"),
    }
    b_d = {
        "q": nc.dram_tensor("bq", [D, 1], f32, kind="ExternalInput"),
        "k": nc.dram_tensor("bk", [D, 1], f32, kind="ExternalInput"),
        "v": nc.dram_tensor("bv", [D, 1], f32, kind="ExternalInput"),
    }
    out_d = nc.dram_tensor("out", [S, D], f32, kind="ExternalOutput")

    with tile.TileContext(nc) as tc:
        with (
            tc.tile_pool(name="const", bufs=1) as cpool,
            tc.tile_pool(name="xt", bufs=1) as xpool,
            tc.tile_pool(name="qkv", bufs=1) as qkvpool,
            tc.tile_pool(name="probs", bufs=20) as ppool,
            tc.tile_pool(name="osb", bufs=2) as opool,
            tc.tile_pool(name="misc", bufs=2) as mpool,
            tc.tile_pool(name="proj_ps", bufs=3, space="PSUM") as proj_ps,
            tc.tile_pool(name="sc_ps", bufs=2, space="PSUM") as sc_ps,
            tc.tile_pool(name="vt_ps", bufs=1, space="PSUM") as vt_ps,
            tc.tile_pool(name="out_ps", bufs=2, space="PSUM") as out_ps,
        ):
            if iters > 1:
                loop_cm = tc.For_i(
                    0,
                    iters,
                    1,
                    hint_engines=(
                        mybir.EngineType.PE,
                        mybir.EngineType.Activation,
                        mybir.EngineType.DVE,
                        mybir.EngineType.SP,
                        mybir.EngineType.Pool,
                    ),
                )
            else:
                loop_cm = contextlib.nullcontext()
            with loop_cm:
                _emit_body(nc, mybir, make_identity, pools={
                    "cpool": cpool, "xpool": xpool, "qkvpool": qkvpool,
                    "ppool": ppool, "opool": opool, "mpool": mpool,
                    "proj_ps": proj_ps, "sc_ps": sc_ps, "vt_ps": vt_ps,
                    "out_ps": out_ps,
                }, dram={
                    "xT": xT_d, "w": w_d, "b": b_d, "out": out_d,
                })

    nc.compile()
    _PROGRAMS[iters] = nc
    return nc


def _emit_body(nc, mybir, make_identity, pools, dram):
    f32 = mybir.dt.float32
    bf16 = mybir.dt.bfloat16
    cpool = pools["cpool"]
    xpool = pools["xpool"]
    qkvpool = pools["qkvpool"]
    ppool = pools["ppool"]
    opool = pools["opool"]
    mpool = pools["mpool"]
    proj_ps = pools["proj_ps"]
    sc_ps = pools["sc_ps"]
    vt_ps = pools["vt_ps"]
    out_ps = pools["out_ps"]
    xT_d = dram["xT"]
    w_d = dram["w"]
    b_d = dram["b"]
    out_d = dram["out"]
    if True:
            # ---- constants ----
            ident = cpool.tile([P, P], bf16, tag="ident")
            make_identity(nc, ident[:])
            # cmaskT[k_local, q_local]: 0 where q >= k (valid), -1e30 where q < k
            cmaskT = cpool.tile([P, P], f32, tag="cmaskT")
            nc.gpsimd.memset(cmaskT[:], 0.0)
            nc.gpsimd.affine_select(
                out=cmaskT[:],
                in_=cmaskT[:],
                compare_op=mybir.AluOpType.is_ge,
                fill=NEG,
                base=0,
                # iota[r, c] = c - r ; keep (0.0) where c - r >= 0
                pattern=[[1, P]],
                channel_multiplier=-1,
            )

            w_sb = {}
            b_sb = {}
            for pj in ("q", "k", "v"):
                w_sb[pj] = cpool.tile([P, NE * D], bf16, name=f"w{pj}", tag=f"w{pj}")
                nc.gpsimd.dma_start(
                    w_sb[pj][:].rearrange("p (ec d) -> p ec d", ec=NE),
                    w_d[pj].rearrange("(ec p) d -> p ec d", p=P),
                )
                b_sb[pj] = cpool.tile([P, 1], f32, name=f"b{pj}", tag=f"b{pj}")
                nc.sync.dma_start(b_sb[pj][:], b_d[pj][:, :])

            # ---- xT load (f32 -> bf16 cast on SWDGE), s-tile-major ----
            xt_sb = xpool.tile([P, NE * S], bf16, tag="xt")
            for st in range(NST):
                nc.gpsimd.dma_start(
                    xt_sb[:].rearrange("p (ec s) -> p ec s", ec=NE)[
                        :, :, st * ST : (st + 1) * ST
                    ],
                    xT_d.rearrange("(ec p) s -> p ec s", p=P)[
                        :, :, st * ST : (st + 1) * ST
                    ],
                )

            qT_sb = qkvpool.tile([P, S], bf16, tag="qT")
            kT_sb = qkvpool.tile([P, S], bf16, tag="kT")
            vT_sb = qkvpool.tile([P, S], bf16, tag="vT")
            v_sb = qkvpool.tile([P, NS * VW], bf16, tag="v")
            dest = {"q": qT_sb, "k": kT_sb, "v": vT_sb}

            # ones column of v_aug
            for sb in range(NS):
                nc.vector.memset(v_sb[:, sb * VW + D : (sb + 1) * VW], 1.0)

            probs_pieces = {}

            for st in range(NST):
                # ---- projections for this s-tile ----
                for pj in ("q", "k", "v"):
                    ps = proj_ps.tile([P, ST], f32, tag="proj")
                    for ec in range(NE):
                        nc.tensor.matmul(
                            ps[:],
                            lhsT=w_sb[pj][:, ec * D : (ec + 1) * D],
                            rhs=xt_sb[:, ec * S + st * ST : ec * S + (st + 1) * ST],
                            start=(ec == 0),
                            stop=(ec == NE - 1),
                        )
                    nc.scalar.activation(
                        dest[pj][:, st * ST : (st + 1) * ST],
                        ps[:],
                        func=mybir.ActivationFunctionType.Identity,
                        bias=b_sb[pj][:, 0:1],
                        scale=1.0,
                    )

                # ---- v natural blocks (PE transpose of vT) ----
                for sb in range(st * (ST // P), (st + 1) * (ST // P)):
                    tp = vt_ps.tile([P, P], bf16, tag="vt")
                    nc.tensor.transpose(
                        tp[:], vT_sb[:, sb * P : (sb + 1) * P], ident[:]
                    )
                    nc.vector.tensor_copy(v_sb[:, sb * VW : sb * VW + D], tp[:])

                # ---- scoresT + exp for q-chunk c = st ----
                c = st
                for j in range(4 * c + 4):
                    qs = max(c * ST, j * P)
                    w = (c + 1) * ST - qs
                    sps = sc_ps.tile([P, ST], f32, tag="sc")
                    nc.tensor.matmul(
                        sps[:, :w],
                        lhsT=kT_sb[:, j * P : (j + 1) * P],
                        rhs=qT_sb[:, qs : qs + w],
                        start=True,
                        stop=True,
                    )
                    if j * P >= c * ST:
                        # diagonal block occupies the first 128 columns
                        nc.vector.tensor_add(sps[:, 0:P], sps[:, 0:P], cmaskT[:])
                    prb = ppool.tile([P, ST], bf16, tag="probs")
                    nc.scalar.activation(
                        prb[:, :w],
                        sps[:, :w],
                        func=mybir.ActivationFunctionType.Exp,
                        bias=0.0,
                        scale=SCALE,
                    )
                    probs_pieces[(j, c)] = (prb, qs)

                # ---- AV + normalize + store for the 4 q-blocks of chunk c ----
                for i in range(4 * c, 4 * c + 4):
                    ops = out_ps.tile([P, VW], f32, tag="out")
                    for j in range(i + 1):
                        prb, qs = probs_pieces[(j, c)]
                        off = i * P - qs
                        nc.tensor.matmul(
                            ops[:],
                            lhsT=prb[:, off : off + P],
                            rhs=v_sb[:, j * VW : (j + 1) * VW],
                            start=(j == 0),
                            stop=(j == i),
                        )
                    recip = mpool.tile([P, 1], f32, tag="recip")
                    nc.vector.reciprocal(recip[:], ops[:, D : D + 1])
                    osb = opool.tile([P, D], f32, tag="osb")
                    nc.scalar.activation(
                        osb[:],
                        ops[:, 0:D],
                        func=mybir.ActivationFunctionType.Copy,
                        bias=0.0,
                        scale=recip[:, 0:1],
                    )
                    nc.sync.dma_start(out_d[i * P : (i + 1) * P, :], osb[:])


def make_in_maps(x, Wq, bq, Wk, bk, Wv, bv):
    x = np.asarray(x, dtype=np.float32)
    shared = {
        "wqT": np.ascontiguousarray(np.asarray(Wq, dtype=np.float32).T),
        "wkT": np.ascontiguousarray(np.asarray(Wk, dtype=np.float32).T),
        "wvT": np.ascontiguousarray(np.asarray(Wv, dtype=np.float32).T),
        "bq": np.asarray(bq, dtype=np.float32).reshape(D, 1).copy(),
        "bk": np.asarray(bk, dtype=np.float32).reshape(D, 1).copy(),
        "bv": np.asarray(bv, dtype=np.float32).reshape(D, 1).copy(),
    }
    return [
        {"xT": np.ascontiguousarray(x[b].T), **shared} for b in range(B)
    ]


def kernel(x, Wq, bq, Wk, bk, Wv, bv):
    from concourse.bass_utils import run_bass_kernel_spmd

    nc = build_program()
    in_maps = make_in_maps(x, Wq, bq, Wk, bk, Wv, bv)
    res = run_bass_kernel_spmd(nc, in_maps, list(range(B)))
    return np.stack([res.results[i]["out"] for i in range(B)], axis=0)
